# revision 33
# baseline (speedup 1.0000x reference)
"""Trainium2 Bass kernel for nn_ClipOTLoss (CLIP-style OT/Sinkhorn loss).

Computes, for full inputs features[B,D], prototypes[K,D], logits[B,K]:
    w = normalize(prototypes, axis=1)
    sims = features @ w.T / TEMPERATURE
    soft_code = sinkhorn(sims)            (3 iters, eps=0.7)
    loss = -mean_b sum_k soft_code * log_softmax(logits)

Distribution: data-parallel over B across 8 NeuronCores; prototypes
replicated; the Sinkhorn row-marginal (sum over B per prototype k)
is a 16KB AllReduce per iteration.  Per-core partial losses are summed
on the host (no final AllReduce).

Design notes (v10, ~360us vs the ~380us v4 baseline):
- Host stages features.T AND prototypes.T as fp8 e4m3 (features
  pre-scaled by SF_F), logits bf16.  Zero device-side casts.
- W is NOT pre-normalized: the main matmul runs on raw fp8 prototypes;
  the per-k norm rn = SF_W/||w_k|| is applied to each PSUM tile by a
  DVE tensor_tensor against a broadcast row before the in-place exp.
  The normalize work rides the MM phase instead of serializing ahead
  of it (v4 spent ~60us there).
- norm2 = ones-matvec over bf16 squares parked in dead E storage;
  rn = exp(-0.5 ln norm2 + ln SF_W) computed in place on the norm2
  PSUM rows (Ln/Exp grouped, Exp writes the bf16 row directly so the
  PSUM frees early), broadcast across partitions per K-half on
  GpSimd partition_broadcast (the PE FIFO never waits on it).
- Main MM is chunk-major fp8 DoubleRow; exp accum gives per-chunk
  colsums; matvec0 follows immediately; ONE 16KB AllReduce per
  Sinkhorn iteration (v4's split-half first AR cost ~10us).
- Dummy max-AllReduce triggered at t~0 from a memset absorbs the
  multi-core launch skew + collectives warmup during the MM phase;
  its result (as a 1.0 gate) is consumed only after the MM loop.
- Sinkhorn iterations: ratio hop = [16,128] spread, DVE recip, PE
  outer-product broadcast, ACT copies out of PSUM; E *= ratio via DVE
  TT with colsum accums split ACT copy-accum (chunks 0-4) / fused STT
  (tail) so the last beta pairs never wait on the ACT chain; beta
  pair updates run on GpSimd (own queue - the DVE scheduler had been
  parking them ~14us late); next iteration's matvec interleaves per
  beta pair.  Dead matmuls drip through AR windows and between
  matvec pairs to hold the PE HAM clock at 2.4GHz.
- Finale: dot'_b and s_b via TT + ACT copy-accum / tail STT; GpSimd
  is kept OFF big tensor ops (a concurrent GpSimd TT was measured to
  slow DVE TTs ~4x).  log_softmax never materialized:
  loss_b = LSE_b - dot'_b/s_b.
- Measured engine costs this kernel is balanced around ([P,4096]
  bf16): DVE TT 2.75us, DVE STT 5.3us (1x), ACT copy-accum 4.4us,
  tensor_scalar+accum 4.4us (1x, not 4x), GpSimd TT ~10us.
"""

import os
import sys

import numpy as np

sys.path.insert(0, "/opt/trn_rl_repo")

import concourse.bass as bass  # noqa: E402
import concourse.bacc as bacc  # noqa: E402
import concourse.tile as tile  # noqa: E402
import concourse.mybir as mybir  # noqa: E402

F32 = mybir.dt.float32
BF16 = mybir.dt.bfloat16
FP8 = mybir.dt.float8e4
AF = mybir.ActivationFunctionType
ALU = mybir.AluOpType
PM = mybir.MatmulPerfMode

TEMPERATURE = 0.01
EPSILON = 0.7
NUM_ITERS = 3
TINY = 1e-8

P = 128
NSLICE = 512
MVS = 1024 if int(os.environ.get("MM1024", "0")) else 512
SF_W = 32.0
SF_F = 256.0


def build_nc(B_loc=1024, K=4096, D=1024, n_cores=8):
    NB = B_loc // P
    ND = D // P
    KH = K // 2
    NKH = (K // 2) // P  # 16 rows of 128 per K-half in the ratio hop
    exp_scale = 1.0 / (TEMPERATURE * EPSILON * SF_W * SF_F)
    r_marg = 1.0 / K
    c_marg = 1.0 / (B_loc * n_cores)
    loss_scale = 1.0 / (B_loc * n_cores)
    rg = [list(range(n_cores))]

    nc = bacc.Bacc(None, target_bir_lowering=False, debug=False)

    f8_d = nc.declare_dram_parameter("f8", [D, B_loc], FP8, isOutput=False)
    wT8 = nc.declare_dram_parameter("wT8", [D, K], FP8, isOutput=False)
    lg_d = nc.declare_dram_parameter("lg", [B_loc, K], BF16, isOutput=False)
    out_ext = nc.declare_dram_parameter("out", [1], F32, isOutput=True)

    m_in_d = [nc.dram_tensor(f"cc_m_in{i}", [K], F32) for i in range(NUM_ITERS)]
    m_out_d = [
        nc.dram_tensor(f"cc_m_out{i}", [K], F32, addr_space="Shared")
        for i in range(NUM_ITERS)
    ]
    d_in_d = nc.dram_tensor("cc_d_in", [P], F32)
    d_out_d = nc.dram_tensor("cc_d_out", [P], F32, addr_space="Shared")

    with tile.TileContext(nc) as tc:
        with (
            tc.tile_pool(name="single", bufs=1) as single,
            tc.tile_pool(name="stage", bufs=2) as stg,
            tc.tile_pool(name="rows", bufs=1) as rows,
            tc.tile_pool(name="big", bufs=1) as bigp,
            tc.tile_pool(name="ps", bufs=2, space="PSUM") as psp,
        ):
            smf = single.tile([P, 400], F32, tag="smf")
            smb = single.tile([P, 1040], BF16, tag="smb")

            class _Cols:
                def __init__(self, t):
                    self.t, self.off = t, 0

                def take(self, np_, nf):
                    ap = self.t[:np_, self.off : self.off + nf]
                    self.off += nf
                    return ap

            cf, cb = _Cols(smf), _Cols(smb)
            ones_col_f = cf.take(P, 1)
            cs_fl = cf.take(P, NB * 2)
            se_fl = cf.take(P, NB)
            se2 = cf.take(P, NB)
            lse = cf.take(P, NB)
            cs0 = cf.take(P, NB)
            beta = cf.take(P, NB)
            tmpb = cf.take(P, NB)
            tmpb2 = cf.take(P, NB)
            vp_fl = cf.take(P, NB * NUM_ITERS)
            rs = cf.take(P, NB)
            dot_fl = cf.take(P, NB)
            dotn = cf.take(P, NB)
            losses = cf.take(P, NB)
            lcol = cf.take(P, 1)
            mg_sb = cf.take(48, P)  # halves at partition 0 and 32
            rt_f = cf.take(48, P)
            loss_sb = cf.take(1, 8)
            dcol = cf.take(P, 1)
            gate = cf.take(P, 1)
            probe_acc = cf.take(P, 1)
            lnw_col = cf.take(P, 1)

            ones_col_bf = cb.take(P, 1)
            beta_bf = cb.take(P, NB)
            rt_bf = cb.take(48, P)  # halves at partition 0 and 32
            ones_row_bf = cb.take(1, P)
            dead_bf = cb.take(P, NSLICE)
            ones_f8 = cb.take(P, 1).bitcast(FP8)  # [P, 2] fp8 ones


            nc.vector.memset(ones_col_f, 1.0)
            nc.vector.memset(ones_col_bf, 1.0)
            nc.vector.memset(ones_row_bf, 1.0)
            nc.vector.memset(dead_bf, 1.0)
            nc.vector.memset(lnw_col, float(np.log(SF_W)))
            nc.vector.memset(ones_f8, 1.0)

            # ---- persistent big tensors ----
            E = bigp.tile([P, NB, K], BF16, tag="E")
            LG = bigp.tile([P, NB, K], BF16, tag="LG")
            WN8 = bigp.tile([P, ND, K], FP8, tag="WN8")
            F8 = bigp.tile([P, ND, B_loc], FP8, tag="F8")
            RNBC = bigp.tile([P, K], BF16, tag="RNBC")
            RBC = RNBC  # reused: rn dead after the MM phase, ratios after
            rn_row = rows.tile([1, K], BF16, tag="row")

            # squares live fp8-packed in dead E storage: chunk j at
            # E[:, j//2, :] bitcast; pair p = E[:, p, :] as [P, 2, 4096]
            sq_flat = E[:, 0:4, :].rearrange("p a k -> p (a k)").bitcast(FP8)
            # dummy-out scratch over WN8 (dead after the main MM)
            act_scr = WN8[:, :, :].rearrange("p a b -> p (a b)").bitcast(BF16)

            # =========================================================
            # Perf probes on dead data (E slots 5-7; first real write
            # to those slots is the main MM's TT-scale much later).
            # Read their durations from the trace; no consumers.
            # =========================================================
            PROBE_TTR = int(os.environ.get("PROBE_TTR", "0"))
            PROBE_GPS = int(os.environ.get("PROBE_GPS", "0"))
            if PROBE_TTR or PROBE_GPS:
                nc.vector.memset(E[:, 6, :], 1.0)
                nc.vector.memset(E[:, 7, :], 0.5)
            if PROBE_TTR:
                nc.vector.tensor_tensor_reduce(
                    out=E[:, 5, :], in0=E[:, 6, :], in1=E[:, 7, :], scale=1.0,
                    scalar=0.0, op0=ALU.mult, op1=ALU.add, accum_out=probe_acc,
                )
            if PROBE_GPS:
                nc.gpsimd.scalar_tensor_tensor(
                    out=E[:, 5, :], in0=E[:, 6, :], scalar=1.0, in1=E[:, 7, :],
                    op0=ALU.mult, op1=ALU.mult, accum_out=probe_acc,
                )

            # =========================================================
            # Input DMAs. Prototypes first (they gate squares/norm2 and
            # the MM), then features, then logits.
            # =========================================================
            wT8v = wT8.rearrange("(j p) k -> p j k", p=P)
            for j in range(ND):
                nc.sync.dma_start(out=WN8[:, j, :], in_=wT8v[:, j, :])
            f8v = f8_d.rearrange("(j p) b -> p j b", p=P)
            for j in range(ND):
                nc.sync.dma_start(out=F8[:, j, :], in_=f8v[:, j, :])
            for c in range(NB):
                nc.sync.dma_start(out=LG[:, c, :], in_=lg_d[c * P : (c + 1) * P, :])

            # dummy skew-absorbing AllReduce, triggered immediately
            # (no input-DMA dependency): absorbs launch skew + warms
            # the collectives path while the MM phase runs.  Its
            # result is consumed (as gate=1) only after the MM loop.
            nc.vector.memset(dcol, 1.0)
            nc.sync.dma_start(out=d_in_d[:], in_=dcol)
            nc.gpsimd.collective_compute(
                "AllReduce",
                ALU.max,
                replica_groups=rg,
                ins=[d_in_d[:]],
                outs=[d_out_d[:]],
            )

            # =========================================================
            # Squares split DVE/ACT by arrival order; even chunks DVE,
            # odd chunks ACT.  DR mode packs them fp8 into E[:,0:4,:];
            # fallback writes bf16 into E[:,j,:].
            # =========================================================
            NORM2_DR = int(os.environ.get("NORM2_DR", "0"))
            for j in range(ND):
                dst = (
                    sq_flat[:, j * K : (j + 1) * K] if NORM2_DR else E[:, j, :]
                )
                if j % 2 == 0:
                    nc.vector.tensor_tensor(
                        out=dst, in0=WN8[:, j, :], in1=WN8[:, j, :], op=ALU.mult
                    )
                else:
                    nc.scalar.activation(out=dst, in_=WN8[:, j, :], func=AF.Square)

            # PE warm-up: dead matmuls so norm2 + MM start at 2.4 GHz
            warm_ps = psp.tile([1, NSLICE], F32, tag="ps")
            for _ in range(12):
                nc.tensor.matmul(
                    warm_ps[:1, :], ones_col_bf[:, :1], dead_bf[:, :],
                    start=True, stop=True,
                )

            # norm2 via fp8 DoubleRow ones-matvec over square pairs
            nv0 = psp.tile([1, KH], F32, tag="ps")
            nv1 = psp.tile([1, KH], F32, tag="ps")
            nv = [nv0, nv1]
            if NORM2_DR:
                for pair in range(4):
                    sqp = E[:, pair, :].bitcast(FP8).rearrange(
                        "p (two k) -> p two k", two=2
                    )
                    for half in range(2):
                        for n in range(KH // NSLICE):
                            s = half * KH + n * NSLICE
                            nc.tensor.matmul(
                                nv[half][:1, n * NSLICE : (n + 1) * NSLICE],
                                ones_f8.rearrange("p (t o) -> p t o", o=1),
                                sqp[:, :, s : s + NSLICE],
                                start=(pair == 0),
                                stop=(pair == 3),
                                perf_mode=PM.DoubleRow,
                            )
            else:
                for j in range(ND):
                    for half in range(2):
                        for n in range(KH // NSLICE):
                            s = half * KH + n * NSLICE
                            nc.tensor.matmul(
                                nv[half][:1, n * NSLICE : (n + 1) * NSLICE],
                                ones_col_bf[:, :1],
                                E[:, j, s : s + NSLICE],
                                start=(j == 0),
                                stop=(j == ND - 1),
                            )
            # rn = exp(-0.5 ln norm2 + ln SF_W), computed in place on
            # the nv PSUM rows (no DMA hops; Ln/Exp grouped so each
            # table loads once), copied to a bf16 row, then broadcast
            # across partitions on GpSimd (the PE FIFO never waits).
            with tc.high_priority():
                for half in range(2):
                    nc.scalar.activation(
                        out=nv[half][:1, :], in_=nv[half][:1, :], func=AF.Ln
                    )
                for half in range(2):
                    nc.scalar.activation(
                        out=rn_row[:1, half * KH : (half + 1) * KH],
                        in_=nv[half][:1, :],
                        func=AF.Exp,
                        scale=-0.5,
                        bias=lnw_col[:1, :1],
                    )
                for half in range(2):
                    nc.gpsimd.partition_broadcast(
                        RNBC[:, half * KH : (half + 1) * KH],
                        rn_row[:1, half * KH : (half + 1) * KH],
                        channels=P,
                    )

            # =========================================================
            # Main matmul, c-major: per chunk both K-halves (fp8 DR),
            # then DVE rn-scale out of PSUM into E, then exp in place
            # with colsum accum.  LSE exps ride the ACT gaps with a
            # one-chunk lag.
            # =========================================================
            def lse_unit(lc, lq):
                lse_scr = stg.tile([P, KH], BF16, tag="stage")
                nc.scalar.activation(
                    out=lse_scr[:, :],
                    in_=LG[:, lc, lq * KH : (lq + 1) * KH],
                    func=AF.Exp,
                    accum_out=(se_fl if lq == 0 else se2)[:, lc : lc + 1],
                )

            for c in range(NB):
                mm0 = psp.tile([P, KH], F32, tag="ps")
                mm1 = psp.tile([P, KH], F32, tag="ps")
                mm = [mm0, mm1]
                for j2 in range(0, ND, 2):
                    for h in range(2):
                        for n in range(KH // MVS):
                            nc.tensor.matmul(
                                mm[h][:, n * MVS : (n + 1) * MVS],
                                F8[:, j2 : j2 + 2, c * P : (c + 1) * P],
                                WN8[
                                    :,
                                    j2 : j2 + 2,
                                    h * KH + n * MVS : h * KH + (n + 1) * MVS,
                                ],
                                start=(j2 == 0),
                                stop=(j2 == ND - 2),
                                perf_mode=PM.DoubleRow,
                            )
                for h in range(2):
                    nc.vector.tensor_tensor(
                        out=E[:, c, h * KH : (h + 1) * KH],
                        in0=mm[h][:, :],
                        in1=RNBC[:, h * KH : (h + 1) * KH],
                        op=ALU.mult,
                    )
                for h in range(2):
                    nc.scalar.activation(
                        out=E[:, c, h * KH : (h + 1) * KH],
                        in_=E[:, c, h * KH : (h + 1) * KH],
                        func=AF.Exp,
                        scale=exp_scale,
                        accum_out=cs_fl[:, c * 2 + h : c * 2 + h + 1],
                    )
                if 1 <= c < NB - 1:
                    lse_unit(c - 1, 0)
                    lse_unit(c - 1, 1)

            # gate = dummy-AR result scaled to 1.0, consumed only here
            # so no engine FIFO waits on the dummy AllReduce mid-MM
            nc.sync.dma_start(
                out=gate, in_=d_out_d[:].rearrange("(a b) -> a b", a=P)
            )
            nc.vector.tensor_scalar(
                out=gate, in0=gate, scalar1=0.0, scalar2=1.0,
                op0=ALU.mult, op1=ALU.add,
            )
            # beta0 = gate / colsum0
            cs_pair = cs_fl.rearrange("p (c two) -> p c two", two=2)
            nc.vector.tensor_tensor(
                out=cs0, in0=cs_pair[:, :, 0], in1=cs_pair[:, :, 1], op=ALU.add
            )
            nc.vector.reciprocal(out=beta, in_=cs0)
            nc.vector.tensor_scalar(
                out=beta, in0=beta, scalar1=gate, scalar2=None, op0=ALU.mult
            )
            nc.vector.tensor_copy(out=beta_bf, in_=beta)

            # =========================================================
            # Sinkhorn.  matvec0 directly after the MM; one 16KB
            # AllReduce per iteration; dead matmuls keep the PE HAM
            # warm through each AllReduce window.
            # =========================================================
            mv0 = psp.tile([1, KH], F32, tag="ps")
            mv1 = psp.tile([1, KH], F32, tag="ps")
            for c in range(NB):
                for half, mv in ((0, mv0), (1, mv1)):
                    for n in range(KH // MVS):
                        nc.tensor.matmul(
                            mv[:1, n * MVS : (n + 1) * MVS],
                            beta_bf[:, c : c + 1],
                            E[
                                :,
                                c,
                                half * KH + n * MVS : half * KH + (n + 1) * MVS,
                            ],
                            start=(c == 0),
                            stop=(c == NB - 1),
                        )

            def emit_mcopy(it, half, mv):
                """m row out of PSUM (halves split DVE/ACT) to DRAM."""
                mr = rows.tile([1, KH], F32, tag="mrow")
                nc.vector.tensor_copy(out=mr[:1, : KH // 2], in_=mv[:1, : KH // 2])
                nc.scalar.copy(out=mr[:1, KH // 2 :], in_=mv[:1, KH // 2 :])
                nc.sync.dma_start(
                    out=m_in_d[it][half * KH : (half + 1) * KH], in_=mr[:1, :]
                )

            def emit_ar(it, mva, mvb):
                for half, mv in ((0, mva), (1, mvb)):
                    emit_mcopy(it, half, mv)
                nc.gpsimd.collective_compute(
                    "AllReduce",
                    ALU.add,
                    replica_groups=rg,
                    ins=[m_in_d[it][:]],
                    outs=[m_out_d[it][:]],
                )

            def emit_warm(n):
                wp = psp.tile([1, NSLICE], F32, tag="ps")
                for _ in range(n):
                    nc.tensor.matmul(
                        wp[:1, :], ones_col_bf[:, :1], dead_bf[:, :],
                        start=True, stop=True,
                    )

            emit_ar(0, mv0, mv1)
            # AR1's window is ~23us (cross-core launch skew); the later
            # ARs are ~8-10us, so this window gets a longer warm drip,
            # and the last four LSE exp units (~8us of ACT) run here
            # instead of crowding the MM phase's ACT tail.
            emit_warm(30)
            for lc in (NB - 2, NB - 1):
                lse_unit(lc, 0)
                lse_unit(lc, 1)
            nc.vector.tensor_tensor(out=se_fl, in0=se_fl, in1=se2, op=ALU.add)
            nc.scalar.activation(out=lse, in_=se_fl, func=AF.Ln)

            def emit_matvec(mv0_, mv1_, cs):
                for c in cs:
                    for half, mv in ((0, mv0_), (1, mv1_)):
                        for n in range(KH // MVS):
                            nc.tensor.matmul(
                                mv[:1, n * MVS : (n + 1) * MVS],
                                beta_bf[:, c : c + 1],
                                E[
                                    :,
                                    c,
                                    half * KH
                                    + n * MVS : half * KH
                                    + (n + 1) * MVS,
                                ],
                                start=(c == 0),
                                stop=(c == NB - 1),
                            )

            N_STT = 2  # chunks NB-N_STT.. use fused STT (prompt tail vp)
            for it in range(NUM_ITERS):
                last = it == NUM_ITERS - 1
                # ---- AllReduce-result hop: ratio in [16,128] form,
                # flatten to a row, broadcast via PE outer product.
                rt1 = rows.tile([1, K], BF16, tag="row")
                for g in range(2):
                    gb = g * 32
                    mg_g = mg_sb[gb : gb + NKH, :]
                    nc.sync.dma_start(
                        out=mg_g,
                        in_=m_out_d[it][g * KH : (g + 1) * KH].rearrange(
                            "(a b) -> a b", a=NKH
                        ),
                    )
                    rb_ps = psp.tile([P, KH], F32, tag="ps")
                    nc.vector.tensor_scalar(
                        out=rt_f[gb : gb + NKH, :], in0=mg_g,
                        scalar1=1.0 / r_marg, scalar2=TINY / r_marg,
                        op0=ALU.mult, op1=ALU.add,
                    )
                    with nc.allow_low_precision(reason="ratio is bf16 anyway"):
                        nc.vector.reciprocal(
                            out=rt_bf[gb : gb + NKH, :],
                            in_=rt_f[gb : gb + NKH, :],
                        )
                    nc.sync.dma_start(
                        out=rt1[:1, g * KH : (g + 1) * KH],
                        in_=rt_bf[gb : gb + NKH, :],
                    )
                    for n in range(KH // NSLICE):
                        nc.tensor.matmul(
                            rb_ps[:, n * NSLICE : (n + 1) * NSLICE],
                            ones_row_bf[:1, :],
                            rt1[:1, g * KH + n * NSLICE : g * KH + (n + 1) * NSLICE],
                            start=True,
                            stop=True,
                        )
                    nc.scalar.copy(
                        out=RBC[:, g * KH : (g + 1) * KH], in_=rb_ps[:, :]
                    )
                if not last:
                    emit_warm(8)

                vp_c = lambda c: vp_fl[:, it * NB + c : it * NB + c + 1]
                if not last:
                    nmv0 = psp.tile([1, KH], F32, tag="ps")
                    nmv1 = psp.tile([1, KH], F32, tag="ps")
                for c in range(NB):
                    # ---- E *= ratio_bc with colsum -> vp: TT + ACT
                    # copy-accum for early chunks, fused STT for the
                    # tail (vp lands on DVE, no ACT-chain wait).
                    # Chunk 0's TT is split by K-half so it starts as
                    # soon as RBC's first half is built.
                    if c == 0:
                        for g in range(2):
                            nc.vector.tensor_tensor(
                                out=E[:, 0, g * KH : (g + 1) * KH],
                                in0=E[:, 0, g * KH : (g + 1) * KH],
                                in1=RBC[:, g * KH : (g + 1) * KH],
                                op=ALU.mult,
                            )
                        nc.scalar.activation(
                            out=act_scr[:, 0:K], in_=E[:, 0, :],
                            func=AF.Copy, accum_out=vp_c(0),
                        )
                    elif c < NB - 2:
                        nc.vector.tensor_tensor(
                            out=E[:, c, :], in0=E[:, c, :], in1=RBC[:, :],
                            op=ALU.mult,
                        )
                        nc.scalar.activation(
                            out=act_scr[:, (c % 2) * K : (c % 2 + 1) * K],
                            in_=E[:, c, :],
                            func=AF.Copy,
                            accum_out=vp_c(c),
                        )
                    else:
                        nc.vector.scalar_tensor_tensor(
                            out=E[:, c, :], in0=E[:, c, :], scalar=1.0,
                            in1=RBC[:, :], op0=ALU.mult, op1=ALU.mult,
                            accum_out=vp_c(c),
                        )
                    if last:
                        # dot'[b] = sum_k Q*logits (1/s applied later)
                        scr = act_scr[:, (2 + c % 2) * K : (3 + c % 2) * K]
                        if c < NB - 2:
                            nc.vector.tensor_tensor(
                                out=scr, in0=E[:, c, :], in1=LG[:, c, :],
                                op=ALU.mult,
                            )
                            nc.scalar.activation(
                                out=scr, in_=scr, func=AF.Copy,
                                accum_out=dot_fl[:, c : c + 1],
                            )
                        else:
                            nc.vector.scalar_tensor_tensor(
                                out=scr, in0=E[:, c, :], scalar=1.0,
                                in1=LG[:, c, :], op0=ALU.mult, op1=ALU.mult,
                                accum_out=dot_fl[:, c : c + 1],
                            )
                    if not last and c % 2 == 1:
                        # beta pair update entirely on GpSimd (its own
                        # queue: never stuck behind the big DVE ops):
                        # beta = (beta*c_marg) / (beta*vp + TINY)
                        pr = slice(c - 1, c + 1)
                        vp_pr = vp_fl[:, it * NB + c - 1 : it * NB + c + 1]
                        nc.gpsimd.tensor_tensor(
                            out=tmpb[:, pr], in0=beta[:, pr], in1=vp_pr,
                            op=ALU.mult,
                        )
                        nc.gpsimd.tensor_scalar(
                            out=tmpb[:, pr], in0=tmpb[:, pr], scalar1=TINY,
                            scalar2=None, op0=ALU.add,
                        )
                        nc.gpsimd.tensor_scalar(
                            out=tmpb2[:, pr], in0=beta[:, pr], scalar1=c_marg,
                            scalar2=None, op0=ALU.mult,
                        )
                        with tc.high_priority():
                            nc.vector.reciprocal(
                                out=tmpb[:, pr], in_=tmpb[:, pr]
                            )
                        nc.gpsimd.tensor_tensor(
                            out=beta[:, pr], in0=tmpb2[:, pr],
                            in1=tmpb[:, pr], op=ALU.mult,
                        )
                        nc.gpsimd.tensor_copy(
                            out=beta_bf[:, pr], in_=beta[:, pr]
                        )
                        emit_matvec(nmv0, nmv1, [c - 1, c])
                        if c < NB - 1:
                            emit_warm(3)
                if not last:
                    emit_ar(it + 1, nmv0, nmv1)
                    emit_warm(14)

            # =========================================================
            # Loss: loss_b = LSE_b - dot'_b / s_b,  s = vp3
            # =========================================================
            nc.vector.reciprocal(
                out=rs, in_=vp_fl[:, (NUM_ITERS - 1) * NB : NUM_ITERS * NB]
            )
            nc.vector.tensor_tensor(out=dotn, in0=dot_fl, in1=rs, op=ALU.mult)
            nc.vector.tensor_tensor(out=losses, in0=lse, in1=dotn, op=ALU.subtract)
            nc.vector.tensor_reduce(
                out=lcol, in_=losses, axis=mybir.AxisListType.X, op=ALU.add
            )
            lp_ps = psp.tile([1, 1], F32, tag="ps")
            nc.tensor.matmul(
                lp_ps[:1, :1], ones_col_f[:, :1], lcol[:, :1], start=True, stop=True
            )
            nc.vector.tensor_scalar(
                out=loss_sb[:1, 0:1], in0=lp_ps[:1, :1], scalar1=loss_scale,
                scalar2=None, op0=ALU.mult,
            )
            nc.sync.dma_start(out=out_ext[:], in_=loss_sb[:1, 0:1])

    nc.compile()
    return nc


LAST_RESULT = None


def kernel(features, prototypes, logits):
    from concourse.bass_utils import run_bass_kernel_spmd
    import ml_dtypes

    global LAST_RESULT
    n_cores = 8
    B, D = features.shape
    K = prototypes.shape[0]
    B_loc = B // n_cores

    nc = build_nc(B_loc=B_loc, K=K, D=D, n_cores=n_cores)

    bf16 = ml_dtypes.bfloat16
    f8 = ml_dtypes.float8_e4m3
    # host staging: shard + transpose + dtype cast (layout/precision
    # prep only; all reference FLOPs run on device)
    wT8_h = np.ascontiguousarray(prototypes.T).astype(f8)
    in_maps = []
    for i in range(n_cores):
        fsl = features[i * B_loc : (i + 1) * B_loc]
        in_maps.append(
            {
                "f8": (np.ascontiguousarray(fsl.T) * SF_F).astype(f8),
                "wT8": wT8_h,
                "lg": logits[i * B_loc : (i + 1) * B_loc].astype(bf16),
            }
        )
    res = run_bass_kernel_spmd(
        nc,
        in_maps,
        list(range(n_cores)),
        trace=bool(os.environ.get("CLIP_OT_TRACE")),
    )
    LAST_RESULT = res
    total = 0.0
    for i in range(n_cores):
        total += float(np.asarray(res.results[i]["out"]).reshape(-1)[0])
    return np.float32(total)


# revision 34
# speedup vs baseline: 1.2728x; 1.2728x over previous
"""Trainium2 Bass kernel for nn_ClipOTLoss (CLIP-style OT/Sinkhorn loss).

Computes, for full inputs features[B,D], prototypes[K,D], logits[B,K]:
    w = normalize(prototypes, axis=1)
    sims = features @ w.T / TEMPERATURE
    soft_code = sinkhorn(sims)            (3 iters, eps=0.7)
    loss = -mean_b sum_k soft_code * log_softmax(logits)

Distribution: data-parallel over B across 8 NeuronCores; prototypes
replicated; the Sinkhorn row-marginal (sum over B per prototype k)
is a 16KB AllReduce per iteration.  Per-core partial losses are summed
on the host (no final AllReduce).

Design notes (v10, ~360us vs the ~380us v4 baseline):
- Host stages features.T AND prototypes.T as fp8 e4m3 (features
  pre-scaled by SF_F), logits bf16.  Zero device-side casts.
- W is NOT pre-normalized: the main matmul runs on raw fp8 prototypes;
  the per-k norm rn = SF_W/||w_k|| is applied to each PSUM tile by a
  DVE tensor_tensor against a broadcast row before the in-place exp.
  The normalize work rides the MM phase instead of serializing ahead
  of it (v4 spent ~60us there).
- norm2 = ones-matvec over bf16 squares parked in dead E storage;
  rn = exp(-0.5 ln norm2 + ln SF_W) computed in place on the norm2
  PSUM rows (Ln/Exp grouped, Exp writes the bf16 row directly so the
  PSUM frees early), broadcast across partitions per K-half on
  GpSimd partition_broadcast (the PE FIFO never waits on it).
- Main MM is chunk-major fp8 DoubleRow; exp accum gives per-chunk
  colsums; matvec0 follows immediately; ONE 16KB AllReduce per
  Sinkhorn iteration (v4's split-half first AR cost ~10us).
- Dummy max-AllReduce triggered at t~0 from a memset absorbs the
  multi-core launch skew + collectives warmup during the MM phase;
  its result (as a 1.0 gate) is consumed only after the MM loop.
- Sinkhorn iterations: ratio hop = [16,128] spread, DVE recip, PE
  outer-product broadcast, ACT copies out of PSUM; E *= ratio via DVE
  TT with colsum accums split ACT copy-accum (chunks 0-4) / fused STT
  (tail) so the last beta pairs never wait on the ACT chain; beta
  pair updates run on GpSimd (own queue - the DVE scheduler had been
  parking them ~14us late); next iteration's matvec interleaves per
  beta pair.  Dead matmuls drip through AR windows and between
  matvec pairs to hold the PE HAM clock at 2.4GHz.
- Finale: dot'_b and s_b via TT + ACT copy-accum / tail STT; GpSimd
  is kept OFF big tensor ops (a concurrent GpSimd TT was measured to
  slow DVE TTs ~4x).  log_softmax never materialized:
  loss_b = LSE_b - dot'_b/s_b.
- Measured engine costs this kernel is balanced around ([P,4096]
  bf16): DVE TT 2.75us, DVE STT 5.3us (1x), ACT copy-accum 4.4us,
  tensor_scalar+accum 4.4us (1x, not 4x), GpSimd TT ~10us.
"""

import os
import sys

import numpy as np

sys.path.insert(0, "/opt/trn_rl_repo")

import concourse.bass as bass  # noqa: E402
import concourse.bacc as bacc  # noqa: E402
import concourse.tile as tile  # noqa: E402
import concourse.mybir as mybir  # noqa: E402

F32 = mybir.dt.float32
BF16 = mybir.dt.bfloat16
FP8 = mybir.dt.float8e4
AF = mybir.ActivationFunctionType
ALU = mybir.AluOpType
PM = mybir.MatmulPerfMode

TEMPERATURE = 0.01
EPSILON = 0.7
NUM_ITERS = 3
TINY = 1e-8

P = 128
NSLICE = 512
MVS = 1024 if int(os.environ.get("MM1024", "0")) else 512
SF_W = 32.0
SF_F = 256.0


def build_nc(B_loc=1024, K=4096, D=1024, n_cores=8):
    NB = B_loc // P
    ND = D // P
    KH = K // 2
    NKH = (K // 2) // P  # 16 rows of 128 per K-half in the ratio hop
    exp_scale = 1.0 / (TEMPERATURE * EPSILON * SF_W * SF_F)
    r_marg = 1.0 / K
    c_marg = 1.0 / (B_loc * n_cores)
    loss_scale = 1.0 / (B_loc * n_cores)
    rg = [list(range(n_cores))]

    nc = bacc.Bacc(None, target_bir_lowering=False, debug=False)

    f8_d = nc.declare_dram_parameter("f8", [D, B_loc], FP8, isOutput=False)
    wT8 = nc.declare_dram_parameter("wT8", [D, K], FP8, isOutput=False)
    lg_d = nc.declare_dram_parameter("lg", [B_loc, K], BF16, isOutput=False)
    out_ext = nc.declare_dram_parameter("out", [1], F32, isOutput=True)

    m_in_d = [nc.dram_tensor(f"cc_m_in{i}", [K], F32) for i in range(NUM_ITERS)]
    m_out_d = [
        nc.dram_tensor(f"cc_m_out{i}", [K], F32, addr_space="Shared")
        for i in range(NUM_ITERS)
    ]
    d_in_d = nc.dram_tensor("cc_d_in", [P], F32)
    d_out_d = nc.dram_tensor("cc_d_out", [P], F32, addr_space="Shared")

    with tile.TileContext(nc) as tc:
        with (
            tc.tile_pool(name="single", bufs=1) as single,
            tc.tile_pool(name="stage", bufs=2) as stg,
            tc.tile_pool(name="rows", bufs=1) as rows,
            tc.tile_pool(name="big", bufs=1) as bigp,
            tc.tile_pool(name="ps", bufs=2, space="PSUM") as psp,
        ):
            smf = single.tile([P, 400], F32, tag="smf")
            smb = single.tile([P, 1040], BF16, tag="smb")

            class _Cols:
                def __init__(self, t):
                    self.t, self.off = t, 0

                def take(self, np_, nf):
                    ap = self.t[:np_, self.off : self.off + nf]
                    self.off += nf
                    return ap

            cf, cb = _Cols(smf), _Cols(smb)
            ones_col_f = cf.take(P, 1)
            cs_fl = cf.take(P, NB * 2)
            se_fl = cf.take(P, NB)
            se2 = cf.take(P, NB)
            lse = cf.take(P, NB)
            cs0 = cf.take(P, NB)
            beta = cf.take(P, NB)
            tmpb = cf.take(P, NB)
            tmpb2 = cf.take(P, NB)
            vp_fl = cf.take(P, NB * NUM_ITERS)
            rs = cf.take(P, NB)
            dot_fl = cf.take(P, NB)
            dotn = cf.take(P, NB)
            losses = cf.take(P, NB)
            lcol = cf.take(P, 1)
            mg_sb = cf.take(48, P)  # halves at partition 0 and 32
            rt_f = cf.take(48, P)
            loss_sb = cf.take(1, 8)
            dcol = cf.take(P, 1)
            gate = cf.take(P, 1)
            probe_acc = cf.take(P, 1)
            lnw_col = cf.take(P, 1)

            ones_col_bf = cb.take(P, 1)
            beta_bf = cb.take(P, NB)
            rt_bf = cb.take(48, P)  # halves at partition 0 and 32
            ones_row_bf = cb.take(1, P)
            dead_bf = cb.take(P, NSLICE)
            ones_f8 = cb.take(P, 1).bitcast(FP8)  # [P, 2] fp8 ones


            nc.vector.memset(ones_col_f, 1.0)
            nc.vector.memset(ones_col_bf, 1.0)
            nc.vector.memset(ones_row_bf, 1.0)
            nc.vector.memset(dead_bf, 1.0)
            nc.vector.memset(lnw_col, float(np.log(SF_W)))
            nc.vector.memset(ones_f8, 1.0)

            # ---- persistent big tensors ----
            E = bigp.tile([P, NB, K], BF16, tag="E")
            LG = bigp.tile([P, NB, K], BF16, tag="LG")
            WN8 = bigp.tile([P, ND, K], FP8, tag="WN8")
            F8 = bigp.tile([P, ND, B_loc], FP8, tag="F8")
            RNBC = bigp.tile([P, K], BF16, tag="RNBC")
            RBC = RNBC  # reused: rn dead after the MM phase, ratios after
            rn_row = rows.tile([1, K], BF16, tag="row")

            # squares live fp8-packed in dead E storage: chunk j at
            # E[:, j//2, :] bitcast; pair p = E[:, p, :] as [P, 2, 4096]
            sq_flat = E[:, 0:4, :].rearrange("p a k -> p (a k)").bitcast(FP8)
            # dummy-out scratch over WN8 (dead after the main MM)
            act_scr = WN8[:, :, :].rearrange("p a b -> p (a b)").bitcast(BF16)

            # =========================================================
            # Perf probes on dead data (E slots 5-7; first real write
            # to those slots is the main MM's TT-scale much later).
            # Read their durations from the trace; no consumers.
            # =========================================================
            PROBE_TTR = int(os.environ.get("PROBE_TTR", "0"))
            PROBE_GPS = int(os.environ.get("PROBE_GPS", "0"))
            if PROBE_TTR or PROBE_GPS:
                nc.vector.memset(E[:, 6, :], 1.0)
                nc.vector.memset(E[:, 7, :], 0.5)
            if PROBE_TTR:
                nc.vector.tensor_tensor_reduce(
                    out=E[:, 5, :], in0=E[:, 6, :], in1=E[:, 7, :], scale=1.0,
                    scalar=0.0, op0=ALU.mult, op1=ALU.add, accum_out=probe_acc,
                )
            if PROBE_GPS:
                nc.gpsimd.scalar_tensor_tensor(
                    out=E[:, 5, :], in0=E[:, 6, :], scalar=1.0, in1=E[:, 7, :],
                    op0=ALU.mult, op1=ALU.mult, accum_out=probe_acc,
                )

            # =========================================================
            # Input DMAs. Prototypes first (they gate squares/norm2 and
            # the MM), then features, then logits.
            # =========================================================
            wT8v = wT8.rearrange("(j p) k -> p j k", p=P)
            for j in range(ND):
                nc.sync.dma_start(out=WN8[:, j, :], in_=wT8v[:, j, :])
            f8v = f8_d.rearrange("(j p) b -> p j b", p=P)
            for j in range(ND):
                nc.sync.dma_start(out=F8[:, j, :], in_=f8v[:, j, :])
            for c in range(NB):
                nc.sync.dma_start(out=LG[:, c, :], in_=lg_d[c * P : (c + 1) * P, :])

            # dummy skew-absorbing AllReduce, triggered immediately
            # (no input-DMA dependency): absorbs launch skew + warms
            # the collectives path while the MM phase runs.  Its
            # result is consumed (as gate=1) only after the MM loop.
            nc.vector.memset(dcol, 1.0)
            nc.sync.dma_start(out=d_in_d[:], in_=dcol)
            nc.gpsimd.collective_compute(
                "AllReduce",
                ALU.max,
                replica_groups=rg,
                ins=[d_in_d[:]],
                outs=[d_out_d[:]],
            )

            # =========================================================
            # Squares split DVE/ACT by arrival order; even chunks DVE,
            # odd chunks ACT.  DR mode packs them fp8 into E[:,0:4,:];
            # fallback writes bf16 into E[:,j,:].
            # =========================================================
            NORM2_DR = int(os.environ.get("NORM2_DR", "0"))
            for j in range(ND):
                dst = (
                    sq_flat[:, j * K : (j + 1) * K] if NORM2_DR else E[:, j, :]
                )
                if j % 2 == 0:
                    nc.vector.tensor_tensor(
                        out=dst, in0=WN8[:, j, :], in1=WN8[:, j, :], op=ALU.mult
                    )
                else:
                    nc.scalar.activation(out=dst, in_=WN8[:, j, :], func=AF.Square)

            # PE warm-up: dead matmuls so norm2 + MM start at 2.4 GHz
            warm_ps = psp.tile([1, NSLICE], F32, tag="ps")
            for _ in range(12):
                nc.tensor.matmul(
                    warm_ps[:1, :], ones_col_bf[:, :1], dead_bf[:, :],
                    start=True, stop=True,
                )

            # norm2 via fp8 DoubleRow ones-matvec over square pairs
            nv0 = psp.tile([1, KH], F32, tag="ps")
            nv1 = psp.tile([1, KH], F32, tag="ps")
            nv = [nv0, nv1]
            if NORM2_DR:
                for pair in range(4):
                    sqp = E[:, pair, :].bitcast(FP8).rearrange(
                        "p (two k) -> p two k", two=2
                    )
                    for half in range(2):
                        for n in range(KH // NSLICE):
                            s = half * KH + n * NSLICE
                            nc.tensor.matmul(
                                nv[half][:1, n * NSLICE : (n + 1) * NSLICE],
                                ones_f8.rearrange("p (t o) -> p t o", o=1),
                                sqp[:, :, s : s + NSLICE],
                                start=(pair == 0),
                                stop=(pair == 3),
                                perf_mode=PM.DoubleRow,
                            )
            else:
                for j in range(ND):
                    for half in range(2):
                        for n in range(KH // NSLICE):
                            s = half * KH + n * NSLICE
                            nc.tensor.matmul(
                                nv[half][:1, n * NSLICE : (n + 1) * NSLICE],
                                ones_col_bf[:, :1],
                                E[:, j, s : s + NSLICE],
                                start=(j == 0),
                                stop=(j == ND - 1),
                            )
            # rn = exp(-0.5 ln norm2 + ln SF_W), computed in place on
            # the nv PSUM rows (no DMA hops; Ln/Exp grouped so each
            # table loads once), copied to a bf16 row, then broadcast
            # across partitions on GpSimd (the PE FIFO never waits).
            with tc.high_priority():
                for half in range(2):
                    nc.scalar.activation(
                        out=nv[half][:1, :], in_=nv[half][:1, :], func=AF.Ln
                    )
                for half in range(2):
                    nc.scalar.activation(
                        out=rn_row[:1, half * KH : (half + 1) * KH],
                        in_=nv[half][:1, :],
                        func=AF.Exp,
                        scale=-0.5,
                        bias=lnw_col[:1, :1],
                    )
                for half in range(2):
                    nc.gpsimd.partition_broadcast(
                        RNBC[:, half * KH : (half + 1) * KH],
                        rn_row[:1, half * KH : (half + 1) * KH],
                        channels=P,
                    )

            # =========================================================
            # Main matmul, c-major: per chunk both K-halves (fp8 DR),
            # then DVE rn-scale out of PSUM into E, then exp in place
            # with colsum accum.  LSE exps ride the ACT gaps with a
            # one-chunk lag.
            # =========================================================
            def lse_unit(lc, lq):
                lse_scr = stg.tile([P, KH], BF16, tag="stage")
                nc.scalar.activation(
                    out=lse_scr[:, :],
                    in_=LG[:, lc, lq * KH : (lq + 1) * KH],
                    func=AF.Exp,
                    accum_out=(se_fl if lq == 0 else se2)[:, lc : lc + 1],
                )

            for c in range(NB):
                mm0 = psp.tile([P, KH], F32, tag="ps")
                mm1 = psp.tile([P, KH], F32, tag="ps")
                mm = [mm0, mm1]
                for j2 in range(0, ND, 2):
                    for h in range(2):
                        for n in range(KH // MVS):
                            nc.tensor.matmul(
                                mm[h][:, n * MVS : (n + 1) * MVS],
                                F8[:, j2 : j2 + 2, c * P : (c + 1) * P],
                                WN8[
                                    :,
                                    j2 : j2 + 2,
                                    h * KH + n * MVS : h * KH + (n + 1) * MVS,
                                ],
                                start=(j2 == 0),
                                stop=(j2 == ND - 2),
                                perf_mode=PM.DoubleRow,
                            )
                for h in range(2):
                    nc.vector.tensor_tensor(
                        out=E[:, c, h * KH : (h + 1) * KH],
                        in0=mm[h][:, :],
                        in1=RNBC[:, h * KH : (h + 1) * KH],
                        op=ALU.mult,
                    )
                for h in range(2):
                    nc.scalar.activation(
                        out=E[:, c, h * KH : (h + 1) * KH],
                        in_=E[:, c, h * KH : (h + 1) * KH],
                        func=AF.Exp,
                        scale=exp_scale,
                        accum_out=cs_fl[:, c * 2 + h : c * 2 + h + 1],
                    )
                if c >= 1:
                    lse_unit(c - 1, 0)
                    lse_unit(c - 1, 1)
            for lc in (NB - 1,):
                lse_unit(lc, 0)
                lse_unit(lc, 1)
            nc.vector.tensor_tensor(out=se_fl, in0=se_fl, in1=se2, op=ALU.add)
            nc.scalar.activation(out=lse, in_=se_fl, func=AF.Ln)

            # gate = dummy-AR result scaled to 1.0, consumed only here
            # so no engine FIFO waits on the dummy AllReduce mid-MM
            nc.sync.dma_start(
                out=gate, in_=d_out_d[:].rearrange("(a b) -> a b", a=P)
            )
            nc.vector.tensor_scalar(
                out=gate, in0=gate, scalar1=0.0, scalar2=1.0,
                op0=ALU.mult, op1=ALU.add,
            )
            # beta0 = gate / colsum0
            cs_pair = cs_fl.rearrange("p (c two) -> p c two", two=2)
            nc.vector.tensor_tensor(
                out=cs0, in0=cs_pair[:, :, 0], in1=cs_pair[:, :, 1], op=ALU.add
            )
            nc.vector.reciprocal(out=beta, in_=cs0)
            nc.vector.tensor_scalar(
                out=beta, in0=beta, scalar1=gate, scalar2=None, op0=ALU.mult
            )
            nc.vector.tensor_copy(out=beta_bf, in_=beta)

            # =========================================================
            # Sinkhorn.  matvec0 directly after the MM; one 16KB
            # AllReduce per iteration; dead matmuls keep the PE HAM
            # warm through each AllReduce window.
            # =========================================================
            mv0 = psp.tile([1, KH], F32, tag="ps")
            mv1 = psp.tile([1, KH], F32, tag="ps")
            for half, mv in ((0, mv0), (1, mv1)):
                for n in range(KH // MVS):
                    for c in range(NB):
                        nc.tensor.matmul(
                            mv[:1, n * MVS : (n + 1) * MVS],
                            beta_bf[:, c : c + 1],
                            E[
                                :,
                                c,
                                half * KH + n * MVS : half * KH + (n + 1) * MVS,
                            ],
                            start=(c == 0),
                            stop=(c == NB - 1),
                        )

            def emit_mcopy(it, half, mv):
                """m row out of PSUM (halves split DVE/ACT) to DRAM."""
                mr = rows.tile([1, KH], F32, tag="mrow")
                nc.vector.tensor_copy(out=mr[:1, : KH // 2], in_=mv[:1, : KH // 2])
                nc.scalar.copy(out=mr[:1, KH // 2 :], in_=mv[:1, KH // 2 :])
                nc.sync.dma_start(
                    out=m_in_d[it][half * KH : (half + 1) * KH], in_=mr[:1, :]
                )

            def emit_ar(it, mva, mvb):
                for half, mv in ((0, mva), (1, mvb)):
                    emit_mcopy(it, half, mv)
                nc.gpsimd.collective_compute(
                    "AllReduce",
                    ALU.add,
                    replica_groups=rg,
                    ins=[m_in_d[it][:]],
                    outs=[m_out_d[it][:]],
                )

            def emit_warm(n):
                wp = psp.tile([1, NSLICE], F32, tag="ps")
                for _ in range(n):
                    nc.tensor.matmul(
                        wp[:1, :], ones_col_bf[:, :1], dead_bf[:, :],
                        start=True, stop=True,
                    )

            emit_ar(0, mv0, mv1)
            # AR1's window is ~23us (cross-core launch skew); the later
            # ARs are ~8-10us, so this window gets a longer warm drip
            emit_warm(30)

            def emit_matvec(mv0_, mv1_, cs):
                for c in cs:
                    for half, mv in ((0, mv0_), (1, mv1_)):
                        for n in range(KH // MVS):
                            nc.tensor.matmul(
                                mv[:1, n * MVS : (n + 1) * MVS],
                                beta_bf[:, c : c + 1],
                                E[
                                    :,
                                    c,
                                    half * KH
                                    + n * MVS : half * KH
                                    + (n + 1) * MVS,
                                ],
                                start=(c == 0),
                                stop=(c == NB - 1),
                            )

            N_STT = 2  # chunks NB-N_STT.. use fused STT (prompt tail vp)
            for it in range(NUM_ITERS):
                last = it == NUM_ITERS - 1
                # ---- AllReduce-result hop: ratio in [16,128] form,
                # flatten to a row, broadcast via PE outer product.
                rt1 = rows.tile([1, K], BF16, tag="row")
                for g in range(2):
                    gb = g * 32
                    mg_g = mg_sb[gb : gb + NKH, :]
                    nc.sync.dma_start(
                        out=mg_g,
                        in_=m_out_d[it][g * KH : (g + 1) * KH].rearrange(
                            "(a b) -> a b", a=NKH
                        ),
                    )
                    rb_ps = psp.tile([P, KH], F32, tag="ps")
                    nc.vector.tensor_scalar(
                        out=rt_f[gb : gb + NKH, :], in0=mg_g,
                        scalar1=1.0 / r_marg, scalar2=TINY / r_marg,
                        op0=ALU.mult, op1=ALU.add,
                    )
                    with nc.allow_low_precision(reason="ratio is bf16 anyway"):
                        nc.vector.reciprocal(
                            out=rt_bf[gb : gb + NKH, :],
                            in_=rt_f[gb : gb + NKH, :],
                        )
                    nc.sync.dma_start(
                        out=rt1[:1, g * KH : (g + 1) * KH],
                        in_=rt_bf[gb : gb + NKH, :],
                    )
                    for n in range(KH // NSLICE):
                        nc.tensor.matmul(
                            rb_ps[:, n * NSLICE : (n + 1) * NSLICE],
                            ones_row_bf[:1, :],
                            rt1[:1, g * KH + n * NSLICE : g * KH + (n + 1) * NSLICE],
                            start=True,
                            stop=True,
                        )
                    nc.scalar.copy(
                        out=RBC[:, g * KH : (g + 1) * KH], in_=rb_ps[:, :]
                    )
                if not last:
                    emit_warm(8)

                vp_c = lambda c: vp_fl[:, it * NB + c : it * NB + c + 1]
                if not last:
                    nmv0 = psp.tile([1, KH], F32, tag="ps")
                    nmv1 = psp.tile([1, KH], F32, tag="ps")
                for c in range(NB):
                    # ---- E *= ratio_bc with colsum -> vp: TT + ACT
                    # copy-accum for early chunks, fused STT for the
                    # tail (vp lands on DVE, no ACT-chain wait).
                    # Chunk 0's TT is split by K-half so it starts as
                    # soon as RBC's first half is built.
                    if c == 0:
                        for g in range(2):
                            nc.vector.tensor_tensor(
                                out=E[:, 0, g * KH : (g + 1) * KH],
                                in0=E[:, 0, g * KH : (g + 1) * KH],
                                in1=RBC[:, g * KH : (g + 1) * KH],
                                op=ALU.mult,
                            )
                        nc.scalar.activation(
                            out=act_scr[:, 0:K], in_=E[:, 0, :],
                            func=AF.Copy, accum_out=vp_c(0),
                        )
                    elif c < NB - 2:
                        nc.vector.tensor_tensor(
                            out=E[:, c, :], in0=E[:, c, :], in1=RBC[:, :],
                            op=ALU.mult,
                        )
                        nc.scalar.activation(
                            out=act_scr[:, (c % 2) * K : (c % 2 + 1) * K],
                            in_=E[:, c, :],
                            func=AF.Copy,
                            accum_out=vp_c(c),
                        )
                    else:
                        nc.vector.scalar_tensor_tensor(
                            out=E[:, c, :], in0=E[:, c, :], scalar=1.0,
                            in1=RBC[:, :], op0=ALU.mult, op1=ALU.mult,
                            accum_out=vp_c(c),
                        )
                    if last:
                        # dot'[b] = sum_k Q*logits (1/s applied later)
                        scr = act_scr[:, (2 + c % 2) * K : (3 + c % 2) * K]
                        if c < NB - 2:
                            nc.vector.tensor_tensor(
                                out=scr, in0=E[:, c, :], in1=LG[:, c, :],
                                op=ALU.mult,
                            )
                            nc.scalar.activation(
                                out=scr, in_=scr, func=AF.Copy,
                                accum_out=dot_fl[:, c : c + 1],
                            )
                        else:
                            nc.vector.scalar_tensor_tensor(
                                out=scr, in0=E[:, c, :], scalar=1.0,
                                in1=LG[:, c, :], op0=ALU.mult, op1=ALU.mult,
                                accum_out=dot_fl[:, c : c + 1],
                            )
                    if not last and c % 2 == 1:
                        # beta pair update entirely on GpSimd (its own
                        # queue: never stuck behind the big DVE ops):
                        # beta = (beta*c_marg) / (beta*vp + TINY)
                        pr = slice(c - 1, c + 1)
                        vp_pr = vp_fl[:, it * NB + c - 1 : it * NB + c + 1]
                        nc.gpsimd.tensor_tensor(
                            out=tmpb[:, pr], in0=beta[:, pr], in1=vp_pr,
                            op=ALU.mult,
                        )
                        nc.gpsimd.tensor_scalar(
                            out=tmpb[:, pr], in0=tmpb[:, pr], scalar1=TINY,
                            scalar2=None, op0=ALU.add,
                        )
                        nc.gpsimd.tensor_scalar(
                            out=tmpb2[:, pr], in0=beta[:, pr], scalar1=c_marg,
                            scalar2=None, op0=ALU.mult,
                        )
                        with tc.high_priority():
                            nc.vector.reciprocal(
                                out=tmpb[:, pr], in_=tmpb[:, pr]
                            )
                        nc.gpsimd.tensor_tensor(
                            out=beta[:, pr], in0=tmpb2[:, pr],
                            in1=tmpb[:, pr], op=ALU.mult,
                        )
                        nc.gpsimd.tensor_copy(
                            out=beta_bf[:, pr], in_=beta[:, pr]
                        )
                        emit_matvec(nmv0, nmv1, [c - 1, c])
                        if c < NB - 1:
                            emit_warm(3)
                if not last:
                    emit_ar(it + 1, nmv0, nmv1)
                    emit_warm(14)

            # =========================================================
            # Loss: loss_b = LSE_b - dot'_b / s_b,  s = vp3
            # =========================================================
            nc.vector.reciprocal(
                out=rs, in_=vp_fl[:, (NUM_ITERS - 1) * NB : NUM_ITERS * NB]
            )
            nc.vector.tensor_tensor(out=dotn, in0=dot_fl, in1=rs, op=ALU.mult)
            nc.vector.tensor_tensor(out=losses, in0=lse, in1=dotn, op=ALU.subtract)
            nc.vector.tensor_reduce(
                out=lcol, in_=losses, axis=mybir.AxisListType.X, op=ALU.add
            )
            lp_ps = psp.tile([1, 1], F32, tag="ps")
            nc.tensor.matmul(
                lp_ps[:1, :1], ones_col_f[:, :1], lcol[:, :1], start=True, stop=True
            )
            nc.vector.tensor_scalar(
                out=loss_sb[:1, 0:1], in0=lp_ps[:1, :1], scalar1=loss_scale,
                scalar2=None, op0=ALU.mult,
            )
            nc.sync.dma_start(out=out_ext[:], in_=loss_sb[:1, 0:1])

    nc.compile()
    return nc


LAST_RESULT = None


def kernel(features, prototypes, logits):
    from concourse.bass_utils import run_bass_kernel_spmd
    import ml_dtypes

    global LAST_RESULT
    n_cores = 8
    B, D = features.shape
    K = prototypes.shape[0]
    B_loc = B // n_cores

    nc = build_nc(B_loc=B_loc, K=K, D=D, n_cores=n_cores)

    bf16 = ml_dtypes.bfloat16
    f8 = ml_dtypes.float8_e4m3
    # host staging: shard + transpose + dtype cast (layout/precision
    # prep only; all reference FLOPs run on device)
    wT8_h = np.ascontiguousarray(prototypes.T).astype(f8)
    in_maps = []
    for i in range(n_cores):
        fsl = features[i * B_loc : (i + 1) * B_loc]
        in_maps.append(
            {
                "f8": (np.ascontiguousarray(fsl.T) * SF_F).astype(f8),
                "wT8": wT8_h,
                "lg": logits[i * B_loc : (i + 1) * B_loc].astype(bf16),
            }
        )
    res = run_bass_kernel_spmd(
        nc,
        in_maps,
        list(range(n_cores)),
        trace=bool(os.environ.get("CLIP_OT_TRACE")),
    )
    LAST_RESULT = res
    total = 0.0
    for i in range(n_cores):
        total += float(np.asarray(res.results[i]["out"]).reshape(-1)[0])
    return np.float32(total)
